# revision 13
# baseline (speedup 1.0000x reference)
"""Trainium2 Bass kernel for nn_Local_align: per-sample dynamic 3x3 conv.

  img = l2norm(vision, axis=C)                              [B,C,H,W]
  tf  = l2norm(text, axis=-1) @ Wt.T + bt                   [B,Nc,out_dim]
  w   = softmax(tf[..., :-1] grouped per (C, 3x3)), b = tf[..., -1]
  out[b] = conv2d_same(img[b], w[b]) + b                    [B,Nc,H,W]

Sharding: data-parallel over batch B=8, one image per NeuronCore.

Numerics: softmax weights are near-uniform (logit sigma ~0.03), so
w = 1/9 + delta with delta tiny. The conv splits into
  out = delta (x) img  +  (1/9) * box3x3(sum_c img)  + bias
The delta conv runs in fp8 e4m3 (both operands scaled x64) using
DoubleRow matmuls (256-channel contraction per instr, 2x PE rate); the
box term is computed in f16/f32 from the unquantized channel sum, which
kills the fp8 quantization error of both w (via delta) and x (via the
exact box). Validated numerically: rel err ~1e-3 vs 2e-2 budget.

Layout: conv output is computed TRANSPOSED — pixels (128/row) in PSUM
partitions, nc=150 streamed as matmul columns — so the 22-wide nc tail
costs nothing extra (PE cycles = streamed cols x k-chunks). Per px-row:
9 DoubleRow matmuls + 2 PE transposes back to [nc, px], assembled into
[*, 512] window tiles for DMA.
"""

import numpy as np

B = 8
C, H, W = 256, 128, 128
NC, KD = 150, 768
KK = 9  # 3x3 taps
OD = C * KK + 1  # 2305
CKK = C * KK
HW = H * W  # 16384
PW = W + 2  # padded row width 130
PH = H + 2
WS = 512  # pixel window = 4 rows
NWIN = HW // WS  # 32
ROWS_PER_WIN = WS // W  # 4
NC0 = 128  # first nc chunk
NC1 = NC - NC0  # 22
KCH = KD // 128  # 6
CCH = C // 128  # 2
XS = 64.0  # fp8 scale for x_hat and delta
# 9-aligned tf windows so per-window softmax group-sums don't cross windows;
# the +1 on the last window covers the bias column (od 2304)
TF_WINS = [(0, 504), (504, 504), (1008, 504), (1512, 504), (2016, 289)]


def _build_program(reps=1, ablate=frozenset()):
    import concourse.bacc as bacc
    import concourse.tile as tile
    from concourse import mybir

    f32 = mybir.dt.float32
    f16 = mybir.dt.float16

    nc = bacc.Bacc("TRN2", target_bir_lowering=False, debug=False)

    vis = nc.dram_tensor("vision", [C, HW], f32, kind="ExternalInput").ap()
    txt = nc.dram_tensor("text", [NC, KD], f32, kind="ExternalInput").ap()
    wtT = nc.dram_tensor("wtT", [KD, OD], f16, kind="ExternalInput").ap()
    btd = nc.dram_tensor("bt", [OD], f16, kind="ExternalInput").ap()
    out = nc.dram_tensor("out", [NC, HW], f32, kind="ExternalOutput").ap()
    aps = (vis, txt, wtT, btd, out)

    with tile.TileContext(nc) as tc:
        if reps == 1:
            _emit_iteration(nc, tc, mybir, aps, ablate)
        else:
            with tc.For_i(0, reps, 1):
                _emit_iteration(nc, tc, mybir, aps, ablate)

    nc.compile()
    return nc


def _emit_iteration(nc, tc, mybir, aps, ablate=frozenset()):
    from contextlib import ExitStack
    from concourse.masks import make_identity

    vis, txt, wtT, btd, out = aps
    f32 = mybir.dt.float32
    f32r = mybir.dt.float32r
    f16 = mybir.dt.float16
    f8 = mybir.dt.float8e4
    MUL = mybir.AluOpType.mult
    ADD = mybir.AluOpType.add
    AX = mybir.ActivationFunctionType
    X = mybir.AxisListType.X
    DRS = mybir.MatmulPerfMode.DoubleRowSwInterleave

    with ExitStack() as ctx:
        singles = ctx.enter_context(tc.tile_pool(name="singles", bufs=1))
        persist = ctx.enter_context(tc.tile_pool(name="persist", bufs=1))
        rawp = ctx.enter_context(tc.tile_pool(name="rawp", bufs=5))
        pssp = ctx.enter_context(tc.tile_pool(name="pss", bufs=2, space="PSUM"))
        psrp = ctx.enter_context(tc.tile_pool(name="psr", bufs=1, space="PSUM"))
        pshp = ctx.enter_context(tc.tile_pool(name="psh", bufs=1, space="PSUM"))

        # ---------------- constants ----------------
        ident32 = singles.tile([128, 128], f32)
        make_identity(nc, ident32)
        ident16 = singles.tile([128, 128], f16)
        make_identity(nc, ident16)
        # ssq matmul lhsT: all-ones scaled 1/(XS*XS) so psum = ssq/4096
        ones16s = singles.tile([128, 128], f16)
        nc.gpsimd.memset(ones16s, 1.0 / (XS * XS))
        # channel-sum rhs columns: ones (f32, raw data) and 1/4096 (f16, ssq)
        onecol = singles.tile([128, 1], f32)
        nc.gpsimd.memset(onecol, 1.0)
        onecol16 = singles.tile([128, 1], f16)
        nc.gpsimd.memset(onecol16, 1.0 / (XS * XS))
        ones16 = singles.tile([1, 128], f16)
        nc.gpsimd.memset(ones16, 1.0)
        bt16 = singles.tile([1, OD], f16)
        nc.sync.dma_start(out=bt16, in_=btd.unsqueeze(0))
        # identJ: anti-diagonal permutation — un-mirrors the px-reversed
        # psum that DoubleRowSwInterleave produces (see conv below)
        identJ = singles.tile([128, 128], f16)
        nc.gpsimd.memset(identJ, 0.0)
        nc.gpsimd.affine_select(
            out=identJ, in_=identJ, compare_op=mybir.AluOpType.not_equal,
            fill=1.0, base=-127, pattern=[[1, 128]], channel_multiplier=1)
        # T3J: mirrored tridiagonal (bands i+j in {126,127,128}) scaled so
        # that after the vertical 3-tap add, boxT = (1/9) box3x3(sum_c x_hat)
        # in the same mirrored px space as the conv psum
        t3v = 1.0 / (9.0 * XS)  # (applied to SmatT which carries a 64x scale)
        identS = singles.tile([128, 128], f16)
        nc.vector.tensor_scalar_mul(identS, identJ, t3v)
        T3 = singles.tile([128, 128], f16)
        nc.vector.tensor_copy(T3, identS)
        nc.vector.tensor_tensor(
            out=T3[:, 0:127], in0=T3[:, 0:127], in1=identS[:, 1:128], op=ADD)
        nc.vector.tensor_tensor(
            out=T3[:, 1:128], in0=T3[:, 1:128], in1=identS[:, 0:127], op=ADD)

        # ---------------- persistent tiles ----------------
        pads = persist.tile([128, PH, PW, CCH], f8, name="pads")
        wT8 = persist.tile([128, KK, CCH, NC], f8, name="wT8")
        t_hatT = persist.tile([128, KCH, NC], f16)
        tfs = [persist.tile([128, OD], f32, name=f"tf{i}") for i in range(2)]
        SmatT = persist.tile([128, H + 2], f16, name="SmatT")
        Hsb = persist.tile([128, H + 2], f16, name="Hsb")
        boxT = persist.tile([128, H], f16, name="boxT")

        nc.gpsimd.memset(pads[:, 0, :, :], 0.0)
        nc.gpsimd.memset(pads[:, PH - 1, :, :], 0.0)
        nc.gpsimd.memset(pads[:, :, 0, :], 0.0)
        nc.gpsimd.memset(pads[:, :, PW - 1, :], 0.0)
        nc.gpsimd.memset(Hsb[:, 0:1], 0.0)
        nc.gpsimd.memset(Hsb[:, H + 1:H + 2], 0.0)

        # ---------------- phase A: text -> conv weights ----------------
        def emit_phase_a(between=None):
            with (
                tc.tile_pool(name="pa", bufs=1) as pa,
                tc.tile_pool(name="pawt", bufs=10) as pawt,
                tc.tile_pool(name="pst", bufs=2, space="PSUM") as pstp,
                tc.tile_pool(name="psw", bufs=1, space="PSUM") as pswp,
            ):
                t_sbs = []
                for i, (n0, cnt) in enumerate([(0, NC0), (NC0, NC1)]):
                    t_sb = pa.tile([128, KD], f32, name=f"t_sb{i}")
                    nc.sync.dma_start(out=t_sb[:cnt], in_=txt[n0:n0 + cnt])
                    t_sbs.append(t_sb)

                tsq = pa.tile([128, KD], f32)
                stat = pa.tile([128, 4], f32)
                for i, (n0, cnt) in enumerate([(0, NC0), (NC0, NC1)]):
                    t_sb = t_sbs[i]
                    nc.scalar.activation(
                        out=tsq[:cnt], in_=t_sb[:cnt], func=AX.Square,
                        accum_out=stat[:cnt, 0:1],
                    )
                    nc.scalar.activation(
                        out=stat[:cnt, 2:3], in_=stat[:cnt, 0:1],
                        func=AX.Abs_reciprocal_sqrt,
                    )
                    nc.vector.tensor_scalar_mul(t_sb[:cnt], t_sb[:cnt], stat[:cnt, 2:3])
                    for k in range(KCH):
                        pst = pstp.tile([128, 128], f32, name="pst", tag="pst")
                        nc.tensor.transpose(
                            pst[:, :cnt], t_sb[:cnt, k * 128:(k + 1) * 128],
                            ident32[:cnt, :cnt],
                        )
                        nc.vector.tensor_copy(t_hatT[:, k, n0:n0 + cnt], pst[:, :cnt])

                if between is not None:
                    between()

                # tf = t_hat @ Wt.T + bt  (fp16 in, fp32 accum; Wt^T slices
                # streamed from DRAM, shared by both nc chunks), with the
                # softmax (exp, group-sum, normalize) pipelined per window
                chunks = [(0, 0, NC0), (1, NC0, NC1)]
                w16s = [
                    pa.tile([128, CKK], f16, name=f"w16_{i}", tag=f"w16_{i}")
                    for i in range(2)
                ]

                def emit_wtrans(i, cc):
                    n0, cnt = [(0, NC0), (NC0, NC1)][i]
                    w16v = w16s[i].rearrange("p (c t) -> p c t", t=KK)
                    for tap in range(KK):
                        pst = pstp.tile([128, 128], f16, name="pst", tag="pst")
                        nc.tensor.transpose(
                            pst[:, :cnt],
                            w16v[:cnt, cc * 128:(cc + 1) * 128, tap],
                            ident16[:cnt, :cnt],
                        )
                        # delta8 = 64*w - 64/9 (fp8, scaled to dodge subnormals)
                        nc.vector.tensor_scalar(
                            out=wT8[:, tap, cc, n0:n0 + cnt],
                            in0=pst[:, :cnt],
                            scalar1=XS, scalar2=-XS / 9.0,
                            op0=MUL, op1=ADD,
                        )

                for wi, (o0, ws) in enumerate(TF_WINS):
                    psws = [
                        pswp.tile([128, 512], f32, tag=f"psw{i}", name=f"psw{i}")
                        for i in range(2)
                    ]
                    for k in range(KCH):
                        wtsl = pawt.tile([128, 512], f16, tag="wtsl")
                        nc.sync.dma_start(
                            out=wtsl[:, :ws],
                            in_=wtT[k * 128:(k + 1) * 128, o0:o0 + ws],
                        )
                        for i, n0, cnt in chunks:
                            nc.tensor.matmul(
                                psws[i][:cnt, :ws],
                                t_hatT[:, k, n0:n0 + cnt],
                                wtsl[:, :ws],
                                start=(k == 0),
                                stop=False,
                            )
                    wse = ws - 1 if o0 + ws > CKK else ws  # exp excl. bias col
                    ngrp = wse // KK
                    for i, n0, cnt in chunks:
                        nc.tensor.matmul(
                            psws[i][:cnt, :ws],
                            ones16[:1, :cnt],
                            bt16[:1, o0:o0 + ws],
                            start=False,
                            stop=True,
                        )
                        nc.scalar.copy(tfs[i][:cnt, o0:o0 + ws], psws[i][:cnt, :ws])
                        tfw = tfs[i][:cnt, o0:o0 + wse]
                        nc.scalar.activation(out=tfw, in_=tfw, func=AX.Exp)
                        tfv = tfw.rearrange("p (c t) -> p c t", t=KK)
                        ssum = pa.tile([128, C], f32, tag=f"ssum{i}", name="ssum")
                        nc.vector.reduce_sum(out=ssum[:cnt, :ngrp], in_=tfv, axis=X)
                        rsum = pa.tile([128, C], f32, tag=f"rsum{i}", name="rsum")
                        nc.vector.reciprocal(rsum[:cnt, :ngrp], ssum[:cnt, :ngrp])
                        nc.vector.tensor_tensor(
                            out=w16s[i][:cnt, o0:o0 + wse].rearrange(
                                "p (c t) -> p c t", t=KK
                            ),
                            in0=tfv,
                            in1=rsum[:cnt, :ngrp].unsqueeze(2).broadcast_to(
                                [cnt, ngrp, KK]
                            ),
                            op=MUL,
                        )
                        # weight transposes as soon as their c-block columns
                        # are done: cc=0 spans tf windows 0-2, cc=1 spans 2-4
                        if wi == 2:
                            emit_wtrans(i, 0)
                        elif wi == 4:
                            emit_wtrans(i, 1)

        # ---------------- vision normalize + channel-sum pipeline ----------
        # front(w): DMA + squares + ssq/chansum matmuls; mid(w): inv, Srow,
        # Srow->SmatT transpose-DMA (lags 1); back(w): pads mult (Pool),
        # Hcol matmul + Hsb copy + boxT(w-1) adds (lags 2).
        sqp = ctx.enter_context(tc.tile_pool(name="sqp", bufs=3))
        nrm = ctx.enter_context(tc.tile_pool(name="nrm", bufs=4))
        srp = ctx.enter_context(tc.tile_pool(name="srp", bufs=2))
        outp = ctx.enter_context(tc.tile_pool(name="outp", bufs=3))
        state = {}  # per-window tiles in flight

        def emit_boxT(w):
            # boxT[:, 4w..4w+3] = Hsb[:, 4w:4w+4]+Hsb[:,4w+1:4w+5]+Hsb[:,4w+2:4w+6]
            r0 = 4 * w
            tmp = srp.tile([128, ROWS_PER_WIN], f16, tag="btmp", name="btmp")
            nc.vector.tensor_tensor(
                out=tmp, in0=Hsb[:, r0:r0 + 4], in1=Hsb[:, r0 + 1:r0 + 5], op=ADD)
            nc.vector.tensor_tensor(
                out=boxT[:, r0:r0 + 4], in0=tmp, in1=Hsb[:, r0 + 2:r0 + 6], op=ADD)

        def front(w):
            ps = pssp.tile([128, WS], f32, tag="pss", name="ps")
            # psbox: cols 0-3 = transposed channel-sum per image row, cols
            # 4-7 = transposed ssq/4096; filled by 1-col matmuls (PE-free),
            # single bank via start-once-then-accumulate-onto-zeros
            psb = psrp.tile([128, 2 * ROWS_PER_WIN], f32, tag="psr", name="psb")
            # both c-chunks in ONE DMA (3D AP)
            rawt = rawp.tile([128, CCH, WS], f32, name="raw", tag="raw")
            if "visdma" not in ablate:
                nc.sync.dma_start(
                    out=rawt,
                    in_=vis.rearrange("(c p) x -> p c x", p=128)[
                        :, :, w * WS:(w + 1) * WS
                    ],
                )
            raws = [rawt[:, cc, :] for cc in range(CCH)]
            if "norm" not in ablate:
                sqs = []
                for cc in range(CCH):
                    sq = sqp.tile([128, WS], f16, name=f"sq{cc}", tag=f"sq{cc}")
                    # Pool: keeps ACT free for inv/epilogue copies
                    nc.gpsimd.tensor_tensor(
                        out=sq, in0=raws[cc], in1=raws[cc], op=MUL)
                    sqs.append(sq)
                # combine c-chunks (f16 2x DVE) so channel-ssq costs 1 matmul
                ssq = sqp.tile([128, WS], f16, name="ssq", tag="ssq")
                nc.vector.tensor_tensor(out=ssq, in0=sqs[0], in1=sqs[1], op=ADD)
                nc.tensor.matmul(ps, ones16s, ssq, start=True, stop=True)
                for r in range(ROWS_PER_WIN):
                    for cc in range(CCH):
                        nc.tensor.matmul(
                            psb[:, r:r + 1],
                            raws[cc][:, r * W:(r + 1) * W],
                            onecol,
                            start=(r == 0 and cc == 0), stop=False,
                            skip_group_check=True,
                        )
                    nc.tensor.matmul(
                        psb[:, 4 + r:5 + r],
                        ssq[:, r * W:(r + 1) * W],
                        onecol16,
                        start=False, stop=(r == ROWS_PER_WIN - 1),
                        skip_group_check=True,
                    )
            state[w] = [raws, ps, psb, None]

        def mid(w):
            if "norm" in ablate:
                return
            raws, ps, psb, _ = state[w]
            # inv = 64/sqrt(ssq) in one ACT op, straight from PSUM
            inv = nrm.tile([128, WS], f32, tag="inv")
            nc.scalar.activation(out=inv, in_=ps, func=AX.Abs_reciprocal_sqrt)
            state[w][3] = inv
            # transposed inv for the box term (tiny ACT op)
            invT = srp.tile([128, ROWS_PER_WIN], f32, tag="invT", name="invT")
            nc.scalar.activation(
                out=invT, in_=psb[:, 4:8], func=AX.Abs_reciprocal_sqrt)
            # SmatT cols = 64 * sum_c x_hat, pixel-major
            r0 = w * ROWS_PER_WIN
            nc.vector.tensor_tensor(
                out=SmatT[:, 1 + r0:5 + r0], in0=psb[:, 0:4], in1=invT, op=MUL)

        def back(w):
            st = state.pop(w)
            if "norm" in ablate:
                return
            raws, ps, psb, inv = st
            r0 = w * ROWS_PER_WIN
            inv_v = inv.rearrange("p (r x) -> p r x", x=W)
            for cc in range(CCH):
                # Pool: pads8 = 64 * x_hat (fp8), DVE stays free for conv epilogue
                nc.gpsimd.tensor_tensor(
                    out=pads[:, 1 + r0:1 + r0 + ROWS_PER_WIN, 1:1 + W, cc],
                    in0=raws[cc].rearrange("p (r x) -> p r x", x=W),
                    in1=inv_v,
                    op=MUL,
                )
            # horizontal 3-tap via tridiag matmul on this window's SmatT cols
            psh = pshp.tile([128, ROWS_PER_WIN], f32, tag="psh", name="psh")
            nc.tensor.matmul(
                psh, T3, SmatT[:, 1 + r0:1 + r0 + 4], start=True, stop=True)
            nc.scalar.copy(Hsb[:, 1 + r0:1 + r0 + 4], psh)
            if w >= 1:
                emit_boxT(w - 1)
            if w == NWIN - 1:
                emit_boxT(w)

        cursor = [0]  # next front window

        def step_norm():
            w = cursor[0]
            if w < NWIN:
                front(w)
            if w >= 1 and w - 1 < NWIN:
                mid(w - 1)
            if w >= 2 and w - 2 < NWIN:
                back(w - 2)
            cursor[0] += 1

        def ensure_norm(upto):
            # guarantee back(w) (and its boxT) emitted for all w <= upto
            while cursor[0] - 3 < min(upto, NWIN - 1):
                step_norm()

        # prologue: get the normalize pipeline moving during phase A's stalls
        def _prologue():
            for _ in range(8):
                step_norm()

        if "phasea" not in ablate:
            emit_phase_a(between=_prologue)
        else:
            _prologue()

        if "conv" in ablate:
            ensure_norm(NWIN - 1)
            return

        # ---------------- conv: transposed fp8 DoubleRow ----------------
        with (
            tc.tile_pool(name="psc", bufs=3, space="PSUM") as pscp,
            tc.tile_pool(name="pst2", bufs=1, space="PSUM") as pst2p,
            tc.tile_pool(name="sb16p", bufs=3) as sb16p,
        ):
            bias0 = tfs[0][:NC0, CKK:CKK + 1]
            bias1 = tfs[1][:NC1, CKK:CKK + 1]
            # out DMAs deferred one window so they never head-of-line block
            # the SP hwdge queue
            pending_outs = []

            def flush_outs():
                for dst0, dst1, src_ap in pending_outs:
                    nc.sync.dma_start(out=out[dst0[0]:dst0[1], dst1[0]:dst1[1]],
                                      in_=src_ap)
                pending_outs.clear()

            ensure_norm(2)
            for g in range(NWIN):
                ensure_norm(g + 2)
                flush_outs()
                # psT: one full PSUM bank; 8 transposes share it via
                # start-once-then-accumulate-onto-zeros
                psT = pst2p.tile([128, ROWS_PER_WIN, 256], f16, tag="psT")
                sbpair = []
                for j in range(2):  # row pairs
                    psc = pscp.tile([128, 2, NC], f32, tag="psc")
                    for jj in range(2):
                        r = 4 * g + 2 * j + jj
                        for dy in range(3):
                            rowflat = pads[:, r + dy].rearrange(
                                "p a b -> p (a b)")
                            for dx in range(3):
                                tap = dy * 3 + dx
                                nc.tensor.matmul(
                                    psc[:, jj, :],
                                    rowflat[:, 2 * dx:2 * dx + 2 * W],
                                    wT8[:, tap, :, :],
                                    start=(jj == 0 and tap == 0),
                                    stop=(tap == KK - 1),
                                    perf_mode=DRS,
                                    skip_group_check=True,
                                )
                    # sb16 = psc/4096 + boxT  (= delta-conv + box/9, final
                    # pre-bias values in f16, pixel-major)
                    sb16 = sb16p.tile([128, 2, NC], f16, tag="sb16")
                    r0 = 4 * g + 2 * j
                    nc.vector.scalar_tensor_tensor(
                        out=sb16,
                        in0=psc,
                        scalar=1.0 / (XS * XS),
                        in1=boxT[:, r0:r0 + 2].unsqueeze(2).broadcast_to(
                            [128, 2, NC]),
                        op0=MUL, op1=ADD,
                    )
                    sbpair.append(sb16)
                for j in range(2):
                    for jj in range(2):
                        sl = 2 * j + jj
                        nc.tensor.matmul(
                            psT[:, sl, 0:128],
                            sbpair[j][:, jj, 0:NC0],
                            identJ,
                            is_transpose=True,
                            start=(sl == 0), stop=(sl == 0),
                            skip_group_check=True,
                        )
                        nc.tensor.matmul(
                            psT[:NC1, sl, 128:128 + 128],
                            sbpair[j][:, jj, NC0:NC],
                            identJ,
                            is_transpose=True,
                            start=False, stop=False,
                            skip_group_check=True,
                        )
                osb = outp.tile([128, ROWS_PER_WIN, W], f32, tag="osb")
                nc.scalar.activation(
                    out=osb, in_=psT[:, :, 0:128], func=AX.Identity, bias=bias0)
                osb1 = outp.tile([NC1, ROWS_PER_WIN, W], f32, tag="osb1")
                nc.vector.tensor_scalar_add(
                    osb1, psT[:NC1, :, 128:256], bias1)
                if "outdma" not in ablate:
                    o0, o1 = g * WS, (g + 1) * WS
                    pending_outs.append(
                        ((0, NC0), (o0, o1), osb.rearrange("p a b -> p (a b)")))
                    pending_outs.append(
                        ((NC0, NC), (o0, o1), osb1.rearrange("p a b -> p (a b)")))
            flush_outs()


_NC_CACHE = {}


def _get_program(reps=1, ablate=frozenset()):
    ablate = frozenset(ablate)
    key = (reps, ablate)
    if key not in _NC_CACHE:
        _NC_CACHE[key] = _build_program(reps, ablate)
    return _NC_CACHE[key]


def _make_in_maps(vision, text, Wt, bt):
    wtT16 = np.ascontiguousarray(Wt.astype(np.float32).T).astype(np.float16)
    bt16 = bt.astype(np.float16)
    in_maps = []
    for b in range(B):
        in_maps.append({
            "vision": np.ascontiguousarray(vision[b].reshape(C, HW)),
            "text": np.ascontiguousarray(text[b, :, 0, :]),
            "wtT": wtT16,
            "bt": bt16,
        })
    return in_maps


def _run(vision, text, Wt, bt, trace=False):
    from concourse.bass_utils import run_bass_kernel_spmd

    nc = _get_program()
    in_maps = _make_in_maps(vision, text, Wt, bt)
    res = run_bass_kernel_spmd(nc, in_maps, list(range(B)), trace=trace)
    outs = np.stack([np.asarray(res.results[b]["out"]).reshape(NC, H, W) for b in range(B)])
    return outs, res


def kernel(vision, text, Wt, bt):
    outs, _ = _run(vision, text, Wt, bt, trace=False)
    return outs


# revision 21
# speedup vs baseline: 1.0880x; 1.0880x over previous
"""Trainium2 Bass kernel for nn_Local_align: per-sample dynamic 3x3 conv.

  img = l2norm(vision, axis=C)                              [B,C,H,W]
  tf  = l2norm(text, axis=-1) @ Wt.T + bt                   [B,Nc,out_dim]
  w   = softmax(tf[..., :-1] grouped per (C, 3x3)), b = tf[..., -1]
  out[b] = conv2d_same(img[b], w[b]) + b                    [B,Nc,H,W]

Sharding: data-parallel over batch B=8, one image per NeuronCore.

Numerics: softmax weights are near-uniform (logit sigma ~0.03), so
w = 1/9 + delta with delta tiny. The conv splits into
  out = delta (x) img  +  (1/9) * box3x3(sum_c img)  + bias
The delta conv runs in fp8 e4m3 (both operands scaled x64) using
DoubleRow matmuls (256-channel contraction per instr, 2x PE rate); the
box term is computed in f16/f32 from the unquantized channel sum, which
kills the fp8 quantization error of both w (via delta) and x (via the
exact box). Validated numerically: rel err ~1e-3 vs 2e-2 budget.

Layout: conv output is computed TRANSPOSED — pixels (128/row) in PSUM
partitions, nc=150 streamed as matmul columns — so the 22-wide nc tail
costs nothing extra (PE cycles = streamed cols x k-chunks). Per px-row:
9 DoubleRow matmuls + 2 PE transposes back to [nc, px], assembled into
[*, 512] window tiles for DMA.
"""

import numpy as np

B = 8
C, H, W = 256, 128, 128
NC, KD = 150, 768
KK = 9  # 3x3 taps
OD = C * KK + 1  # 2305
CKK = C * KK
HW = H * W  # 16384
PW = W + 2  # padded row width 130
PH = H + 2
WS = 512  # pixel window = 4 rows
NWIN = HW // WS  # 32
ROWS_PER_WIN = WS // W  # 4
NC0 = 128  # first nc chunk
NC1 = NC - NC0  # 22
KCH = KD // 128  # 6
CCH = C // 128  # 2
XS = 64.0  # fp8 scale for x_hat and delta
# 9-aligned tf windows so per-window softmax group-sums don't cross windows;
# the +1 on the last window covers the bias column (od 2304)
TF_WINS = [(0, 504), (504, 504), (1008, 504), (1512, 504), (2016, 289)]


def _build_program(reps=1, ablate=frozenset()):
    import concourse.bacc as bacc
    import concourse.tile as tile
    from concourse import mybir

    f32 = mybir.dt.float32
    f16 = mybir.dt.float16

    nc = bacc.Bacc("TRN2", target_bir_lowering=False, debug=False)

    vis = nc.dram_tensor("vision", [C, HW], f32, kind="ExternalInput").ap()
    txt = nc.dram_tensor("text", [NC, KD], f32, kind="ExternalInput").ap()
    wtT = nc.dram_tensor("wtT", [KD, OD], f16, kind="ExternalInput").ap()
    btd = nc.dram_tensor("bt", [OD], f16, kind="ExternalInput").ap()
    out = nc.dram_tensor("out", [NC, HW], f32, kind="ExternalOutput").ap()
    aps = (vis, txt, wtT, btd, out)

    with tile.TileContext(nc) as tc:
        if reps == 1:
            _emit_iteration(nc, tc, mybir, aps, ablate)
        else:
            with tc.For_i(0, reps, 1):
                _emit_iteration(nc, tc, mybir, aps, ablate)

    nc.compile()
    return nc


def _emit_iteration(nc, tc, mybir, aps, ablate=frozenset()):
    from contextlib import ExitStack
    from concourse.masks import make_identity

    vis, txt, wtT, btd, out = aps
    f32 = mybir.dt.float32
    f32r = mybir.dt.float32r
    f16 = mybir.dt.float16
    f8 = mybir.dt.float8e4
    MUL = mybir.AluOpType.mult
    ADD = mybir.AluOpType.add
    AX = mybir.ActivationFunctionType
    X = mybir.AxisListType.X
    DRS = mybir.MatmulPerfMode.DoubleRowSwInterleave

    with ExitStack() as ctx:
        singles = ctx.enter_context(tc.tile_pool(name="singles", bufs=1))
        persist = ctx.enter_context(tc.tile_pool(name="persist", bufs=1))
        rawp = ctx.enter_context(tc.tile_pool(name="rawp", bufs=6))
        pssp = ctx.enter_context(tc.tile_pool(name="pss", bufs=2, space="PSUM"))
        psrp = ctx.enter_context(tc.tile_pool(name="psr", bufs=2, space="PSUM"))

        # ---------------- constants ----------------
        ident32 = singles.tile([128, 128], f32)
        make_identity(nc, ident32)
        ident16 = singles.tile([128, 128], f16)
        make_identity(nc, ident16)
        # ssq matmul lhsT: all-ones scaled 1/(XS*XS) so psum = ssq/4096
        ones16s = singles.tile([128, 128], f16)
        nc.gpsimd.memset(ones16s, 1.0 / (XS * XS))
        # channel-sum rhs columns: ones (f32, raw data) and 1/4096 (f16, ssq)
        onecol = singles.tile([128, 1], f32)
        nc.gpsimd.memset(onecol, 1.0)
        onecol16 = singles.tile([128, 1], f16)
        nc.gpsimd.memset(onecol16, 1.0 / (XS * XS))
        ones16 = singles.tile([1, 128], f16)
        nc.gpsimd.memset(ones16, 1.0)
        bt16 = singles.tile([1, OD], f16)
        nc.sync.dma_start(out=bt16, in_=btd.unsqueeze(0))
        # identJ: anti-diagonal permutation — un-mirrors the px-reversed
        # psum that DoubleRowSwInterleave produces (see conv below)
        identJ = singles.tile([128, 128], f16)
        nc.gpsimd.memset(identJ, 0.0)
        nc.gpsimd.affine_select(
            out=identJ, in_=identJ, compare_op=mybir.AluOpType.not_equal,
            fill=1.0, base=-127, pattern=[[1, 128]], channel_multiplier=1)
        # T3J: mirrored tridiagonal (bands i+j in {126,127,128}) scaled so
        # that after the vertical 3-tap add, boxT = (1/9) box3x3(sum_c x_hat)
        # in the same mirrored px space as the conv psum
        t3v = 1.0 / (9.0 * XS)  # (applied to SmatT which carries a 64x scale)
        identS = singles.tile([128, 128], f16)
        nc.vector.tensor_scalar_mul(identS, identJ, t3v)
        T3 = singles.tile([128, 128], f16)
        nc.vector.tensor_copy(T3, identS)
        nc.vector.tensor_tensor(
            out=T3[:, 0:127], in0=T3[:, 0:127], in1=identS[:, 1:128], op=ADD)
        nc.vector.tensor_tensor(
            out=T3[:, 1:128], in0=T3[:, 1:128], in1=identS[:, 0:127], op=ADD)

        # ---------------- persistent tiles ----------------
        pads = persist.tile([128, PH, PW, CCH], f8, name="pads")
        wT8 = persist.tile([128, KK, CCH, NC], f8, name="wT8")
        t_hatT = persist.tile([128, KCH, NC], f16)
        tfs = [persist.tile([128, OD], f32, name=f"tf{i}") for i in range(2)]
        SmatT = persist.tile([128, H + 2], f16, name="SmatT")
        Hsb = persist.tile([128, H + 2], f16, name="Hsb")
        boxT = persist.tile([128, H], f16, name="boxT")

        nc.vector.memset(pads[:, 0, :, :], 0.0)
        nc.vector.memset(pads[:, PH - 1, :, :], 0.0)
        nc.vector.memset(pads[:, :, 0, :], 0.0)
        nc.vector.memset(pads[:, :, PW - 1, :], 0.0)
        nc.vector.memset(Hsb[:, 0:1], 0.0)
        nc.vector.memset(Hsb[:, H + 1:H + 2], 0.0)

        # ---------------- phase A: text -> conv weights ----------------
        def emit_phase_a(between=None):
            with (
                tc.tile_pool(name="pa", bufs=1) as pa,
                tc.tile_pool(name="pawt", bufs=10) as pawt,
                tc.tile_pool(name="pst", bufs=2, space="PSUM") as pstp,
                tc.tile_pool(name="psw", bufs=1, space="PSUM") as pswp,
            ):
                t_sbs = []
                for i, (n0, cnt) in enumerate([(0, NC0), (NC0, NC1)]):
                    t_sb = pa.tile([128, KD], f32, name=f"t_sb{i}")
                    nc.sync.dma_start(out=t_sb[:cnt], in_=txt[n0:n0 + cnt])
                    t_sbs.append(t_sb)

                tsq = pa.tile([128, KD], f32)
                stat = pa.tile([128, 4], f32)
                for i, (n0, cnt) in enumerate([(0, NC0), (NC0, NC1)]):
                    t_sb = t_sbs[i]
                    nc.scalar.activation(
                        out=tsq[:cnt], in_=t_sb[:cnt], func=AX.Square,
                        accum_out=stat[:cnt, 0:1],
                    )
                    nc.scalar.activation(
                        out=stat[:cnt, 2:3], in_=stat[:cnt, 0:1],
                        func=AX.Abs_reciprocal_sqrt,
                    )
                    nc.vector.tensor_scalar_mul(t_sb[:cnt], t_sb[:cnt], stat[:cnt, 2:3])
                    for k in range(KCH):
                        pst = pstp.tile([128, 128], f32, name="pst", tag="pst")
                        nc.tensor.transpose(
                            pst[:, :cnt], t_sb[:cnt, k * 128:(k + 1) * 128],
                            ident32[:cnt, :cnt],
                        )
                        nc.vector.tensor_copy(t_hatT[:, k, n0:n0 + cnt], pst[:, :cnt])

                if between is not None:
                    between()

                # tf = t_hat @ Wt.T + bt  (fp16 in, fp32 accum; Wt^T slices
                # streamed from DRAM, shared by both nc chunks), with the
                # softmax (exp, group-sum, normalize) pipelined per window
                chunks = [(0, 0, NC0), (1, NC0, NC1)]
                w16s = [
                    pa.tile([128, CKK], f16, name=f"w16_{i}", tag=f"w16_{i}")
                    for i in range(2)
                ]

                def emit_wtrans(i, cc):
                    n0, cnt = [(0, NC0), (NC0, NC1)][i]
                    w16v = w16s[i].rearrange("p (c t) -> p c t", t=KK)
                    for tap in range(KK):
                        pst = pstp.tile([128, 128], f16, name="pst", tag="pst")
                        nc.tensor.transpose(
                            pst[:, :cnt],
                            w16v[:cnt, cc * 128:(cc + 1) * 128, tap],
                            ident16[:cnt, :cnt],
                        )
                        # delta8 = 64*w - 64/9 (fp8, scaled to dodge subnormals)
                        nc.vector.tensor_scalar(
                            out=wT8[:, tap, cc, n0:n0 + cnt],
                            in0=pst[:, :cnt],
                            scalar1=XS, scalar2=-XS / 9.0,
                            op0=MUL, op1=ADD,
                        )

                for wi, (o0, ws) in enumerate(TF_WINS):
                    psws = [
                        pswp.tile([128, 512], f32, tag=f"psw{i}", name=f"psw{i}")
                        for i in range(2)
                    ]
                    for k in range(KCH):
                        wtsl = pawt.tile([128, 512], f16, tag="wtsl")
                        nc.sync.dma_start(
                            out=wtsl[:, :ws],
                            in_=wtT[k * 128:(k + 1) * 128, o0:o0 + ws],
                        )
                        for i, n0, cnt in chunks:
                            nc.tensor.matmul(
                                psws[i][:cnt, :ws],
                                t_hatT[:, k, n0:n0 + cnt],
                                wtsl[:, :ws],
                                start=(k == 0),
                                stop=False,
                            )
                    wse = ws - 1 if o0 + ws > CKK else ws  # exp excl. bias col
                    ngrp = wse // KK
                    for i, n0, cnt in chunks:
                        nc.tensor.matmul(
                            psws[i][:cnt, :ws],
                            ones16[:1, :cnt],
                            bt16[:1, o0:o0 + ws],
                            start=False,
                            stop=True,
                        )
                        nc.scalar.copy(tfs[i][:cnt, o0:o0 + ws], psws[i][:cnt, :ws])
                        tfw = tfs[i][:cnt, o0:o0 + wse]
                        nc.scalar.activation(out=tfw, in_=tfw, func=AX.Exp)
                        tfv = tfw.rearrange("p (c t) -> p c t", t=KK)
                        ssum = pa.tile([128, C], f32, tag=f"ssum{i}", name="ssum")
                        nc.vector.reduce_sum(out=ssum[:cnt, :ngrp], in_=tfv, axis=X)
                        rsum = pa.tile([128, C], f32, tag=f"rsum{i}", name="rsum")
                        nc.vector.reciprocal(rsum[:cnt, :ngrp], ssum[:cnt, :ngrp])
                        nc.vector.tensor_tensor(
                            out=w16s[i][:cnt, o0:o0 + wse].rearrange(
                                "p (c t) -> p c t", t=KK
                            ),
                            in0=tfv,
                            in1=rsum[:cnt, :ngrp].unsqueeze(2).broadcast_to(
                                [cnt, ngrp, KK]
                            ),
                            op=MUL,
                        )
                        # weight transposes as soon as their c-block columns
                        # are done: cc=0 spans tf windows 0-2, cc=1 spans 2-4
                        if wi == 2:
                            emit_wtrans(i, 0)
                        elif wi == 4:
                            emit_wtrans(i, 1)

        # ---------------- vision normalize + channel-sum pipeline ----------
        # front(w): DMA + squares + ssq/chansum matmuls; mid(w): inv, Srow,
        # Srow->SmatT transpose-DMA (lags 1); back(w): pads mult (Pool),
        # Hcol matmul + Hsb copy + boxT(w-1) adds (lags 2).
        sqp = ctx.enter_context(tc.tile_pool(name="sqp", bufs=3))
        nrm = ctx.enter_context(tc.tile_pool(name="nrm", bufs=4))
        srp = ctx.enter_context(tc.tile_pool(name="srp", bufs=2))
        outp = ctx.enter_context(tc.tile_pool(name="outp", bufs=3))
        state = {}  # per-window tiles in flight

        def emit_boxT(w):
            # boxT[:, 4w..4w+3] = Hsb[:, 4w:4w+4]+Hsb[:,4w+1:4w+5]+Hsb[:,4w+2:4w+6]
            r0 = 4 * w
            tmp = srp.tile([128, ROWS_PER_WIN], f16, tag="btmp", name="btmp")
            nc.vector.tensor_tensor(
                out=tmp, in0=Hsb[:, r0:r0 + 4], in1=Hsb[:, r0 + 1:r0 + 5], op=ADD)
            nc.vector.tensor_tensor(
                out=boxT[:, r0:r0 + 4], in0=tmp, in1=Hsb[:, r0 + 2:r0 + 6], op=ADD)

        def front(w):
            ps = pssp.tile([128, WS], f32, tag="pss", name="ps")
            # psbox: cols 0-3 = transposed channel-sum per image row, cols
            # 4-7 = transposed ssq/4096; filled by 1-col matmuls (PE-free),
            # single bank via start-once-then-accumulate-onto-zeros
            psb = psrp.tile([128, 3 * ROWS_PER_WIN], f32, tag="psr", name="psb")
            # both c-chunks in ONE DMA (3D AP)
            rawt = rawp.tile([128, CCH, WS], f32, name="raw", tag="raw")
            if "visdma" not in ablate:
                nc.sync.dma_start(
                    out=rawt,
                    in_=vis.rearrange("(c p) x -> p c x", p=128)[
                        :, :, w * WS:(w + 1) * WS
                    ],
                )
            raws = [rawt[:, cc, :] for cc in range(CCH)]
            if "norm" not in ablate:
                sq = sqp.tile([128, CCH, WS], f16, name="sq", tag="sq")
                nc.scalar.square(sq, rawt)
                # combine c-chunks (f16 2x DVE) so channel-ssq costs 1 matmul
                ssq = sqp.tile([128, WS], f16, name="ssq", tag="ssq")
                nc.vector.tensor_tensor(
                    out=ssq, in0=sq[:, 0, :], in1=sq[:, 1, :], op=ADD)
                nc.tensor.matmul(ps, ones16s, ssq, start=True, stop=True)
                for r in range(ROWS_PER_WIN):
                    for cc in range(CCH):
                        nc.tensor.matmul(
                            psb[:, r:r + 1],
                            rawt[:, cc, r * W:(r + 1) * W],
                            onecol,
                            start=(r == 0 and cc == 0), stop=False,
                            skip_group_check=True,
                        )
                    nc.tensor.matmul(
                        psb[:, 4 + r:5 + r],
                        ssq[:, r * W:(r + 1) * W],
                        onecol16,
                        start=False, stop=(r == ROWS_PER_WIN - 1),
                        skip_group_check=True,
                    )
            state[w] = [rawt, ps, psb, None]

        def mid(w):
            if "norm" in ablate:
                return
            rawt, ps, psb, _ = state[w]
            # inv = 64/sqrt(ssq) in one ACT op, straight from PSUM
            inv = nrm.tile([128, WS], f32, tag="inv")
            nc.scalar.activation(out=inv, in_=ps, func=AX.Abs_reciprocal_sqrt)
            state[w][3] = inv
            # transposed inv for the box term (tiny ACT op)
            invT = srp.tile([128, ROWS_PER_WIN], f32, tag="invT", name="invT")
            nc.scalar.activation(
                out=invT, in_=psb[:, 4:8], func=AX.Abs_reciprocal_sqrt)
            # SmatT cols = 64 * sum_c x_hat, pixel-major
            r0 = w * ROWS_PER_WIN
            nc.vector.tensor_tensor(
                out=SmatT[:, 1 + r0:5 + r0], in0=psb[:, 0:4], in1=invT, op=MUL)

        def back(w):
            st = state.pop(w)
            if "norm" in ablate:
                return
            rawt, ps, psb, inv = st
            r0 = w * ROWS_PER_WIN
            inv_v = inv.rearrange("p (r x) -> p r x", x=W)
            nc.vector.tensor_tensor(
                out=pads[:, 1 + r0:1 + r0 + ROWS_PER_WIN, 1:1 + W, :],
                in0=rawt.rearrange("p c (r x) -> p r x c", x=W),
                in1=inv_v.unsqueeze(3).broadcast_to(
                    [128, ROWS_PER_WIN, W, CCH]),
                op=MUL,
            )
            # horizontal 3-tap via tridiag matmul on this window's SmatT cols;
            # reuses psb's bank (start=True re-zeroes it, which is safe: the
            # rhs dep on SmatT orders this after every read of psb cols 0-8)
            psh = psb[:, 8:12]
            nc.tensor.matmul(
                psh, T3, SmatT[:, 1 + r0:1 + r0 + 4], start=True, stop=True,
                skip_group_check=True)
            nc.scalar.copy(Hsb[:, 1 + r0:1 + r0 + 4], psh)
            if w >= 1:
                emit_boxT(w - 1)
            if w == NWIN - 1:
                emit_boxT(w)

        cursor = [0]  # next front window

        def step_norm():
            w = cursor[0]
            if w < NWIN:
                front(w)
            if w >= 1 and w - 1 < NWIN:
                mid(w - 1)
            if w >= 2 and w - 2 < NWIN:
                back(w - 2)
            cursor[0] += 1

        def ensure_norm(upto):
            # guarantee back(w) (and its boxT) emitted for all w <= upto
            while cursor[0] - 3 < min(upto, NWIN - 1):
                step_norm()

        # prologue: get the normalize pipeline moving during phase A's stalls
        def _prologue():
            for _ in range(8):
                step_norm()

        if "phasea" not in ablate:
            emit_phase_a(between=_prologue)
        else:
            _prologue()

        if "conv" in ablate:
            ensure_norm(NWIN - 1)
            return

        # ---------------- conv: transposed fp8 DoubleRow ----------------
        with (
            tc.tile_pool(name="psc", bufs=3, space="PSUM") as pscp,
            tc.tile_pool(name="pst2", bufs=1, space="PSUM") as pst2p,
            tc.tile_pool(name="sb16p", bufs=3) as sb16p,
        ):
            bias0 = tfs[0][:NC0, CKK:CKK + 1]
            bias1 = tfs[1][:NC1, CKK:CKK + 1]
            # out DMAs deferred one window so they never head-of-line block
            # the SP hwdge queue
            pending_outs = []

            def flush_outs():
                for dst0, dst1, src_ap in pending_outs:
                    nc.sync.dma_start(out=out[dst0[0]:dst0[1], dst1[0]:dst1[1]],
                                      in_=src_ap)
                pending_outs.clear()

            ensure_norm(2)
            for g in range(NWIN):
                ensure_norm(g + 2)
                flush_outs()
                # psT: one full PSUM bank; 8 transposes share it via
                # start-once-then-accumulate-onto-zeros
                psT = pst2p.tile([128, ROWS_PER_WIN, 256], f16, tag="psT")
                sbpair = []
                for j in range(2):  # row pairs
                    psc = pscp.tile([128, 2, NC], f32, tag="psc")
                    for jj in range(2):
                        r = 4 * g + 2 * j + jj
                        for dy in range(3):
                            rowflat = pads[:, r + dy].rearrange(
                                "p a b -> p (a b)")
                            for dx in range(3):
                                tap = dy * 3 + dx
                                nc.tensor.matmul(
                                    psc[:, jj, :],
                                    rowflat[:, 2 * dx:2 * dx + 2 * W],
                                    wT8[:, tap, :, :],
                                    start=(jj == 0 and tap == 0),
                                    stop=(tap == KK - 1),
                                    perf_mode=DRS,
                                    skip_group_check=True,
                                )
                    # sb16 = psc/4096 + boxT  (= delta-conv + box/9, final
                    # pre-bias values in f16, pixel-major)
                    sb16 = sb16p.tile([128, 2, NC], f16, tag="sb16")
                    r0 = 4 * g + 2 * j
                    nc.vector.scalar_tensor_tensor(
                        out=sb16,
                        in0=psc,
                        scalar=1.0 / (XS * XS),
                        in1=boxT[:, r0:r0 + 2].unsqueeze(2).broadcast_to(
                            [128, 2, NC]),
                        op0=MUL, op1=ADD,
                    )
                    sbpair.append(sb16)
                for j in range(2):
                    for jj in range(2):
                        sl = 2 * j + jj
                        nc.tensor.matmul(
                            psT[:, sl, 0:128],
                            sbpair[j][:, jj, 0:NC0],
                            identJ,
                            is_transpose=True,
                            start=(sl == 0), stop=(sl == 0),
                            skip_group_check=True,
                        )
                        nc.tensor.matmul(
                            psT[:NC1, sl, 128:128 + 128],
                            sbpair[j][:, jj, NC0:NC],
                            identJ,
                            is_transpose=True,
                            start=False, stop=False,
                            skip_group_check=True,
                        )
                osb = outp.tile([128, ROWS_PER_WIN, W], f32, tag="osb")
                nc.scalar.activation(
                    out=osb, in_=psT[:, :, 0:128], func=AX.Identity, bias=bias0)
                osb1 = outp.tile([NC1, ROWS_PER_WIN, W], f32, tag="osb1")
                nc.vector.tensor_scalar_add(
                    osb1, psT[:NC1, :, 128:256], bias1)
                if "outdma" not in ablate:
                    o0, o1 = g * WS, (g + 1) * WS
                    pending_outs.append(
                        ((0, NC0), (o0, o1), osb.rearrange("p a b -> p (a b)")))
                    pending_outs.append(
                        ((NC0, NC), (o0, o1), osb1.rearrange("p a b -> p (a b)")))
            flush_outs()


_NC_CACHE = {}


def _get_program(reps=1, ablate=frozenset()):
    ablate = frozenset(ablate)
    key = (reps, ablate)
    if key not in _NC_CACHE:
        _NC_CACHE[key] = _build_program(reps, ablate)
    return _NC_CACHE[key]


def _make_in_maps(vision, text, Wt, bt):
    wtT16 = np.ascontiguousarray(Wt.astype(np.float32).T).astype(np.float16)
    bt16 = bt.astype(np.float16)
    in_maps = []
    for b in range(B):
        in_maps.append({
            "vision": np.ascontiguousarray(vision[b].reshape(C, HW)),
            "text": np.ascontiguousarray(text[b, :, 0, :]),
            "wtT": wtT16,
            "bt": bt16,
        })
    return in_maps


def _run(vision, text, Wt, bt, trace=False):
    from concourse.bass_utils import run_bass_kernel_spmd

    nc = _get_program()
    in_maps = _make_in_maps(vision, text, Wt, bt)
    res = run_bass_kernel_spmd(nc, in_maps, list(range(B)), trace=trace)
    outs = np.stack([np.asarray(res.results[b]["out"]).reshape(NC, H, W) for b in range(B)])
    return outs, res


def kernel(vision, text, Wt, bt):
    outs, _ = _run(vision, text, Wt, bt, trace=False)
    return outs
